# revision 7
# baseline (speedup 1.0000x reference)
"""Trainium2 Bass kernel for the 8-relation GNN (gnn_message_passing).

Reference computation (see problem):
    h0 = x @ W_in + b_in
    for l in 0..1:
        mean_k = segment_mean(h[src_k], dst_k)   for k in 0..7
        h = relu(concat([h, mean_0..mean_7]) @ W_conv[l] + b_conv[l])
    cat = concat([h0, h1, h2]); y = relu(cat@W_cls1+b1) @ W_cls2 + b2
    out = log_softmax(y)

Strategy (8 NeuronCores, node sharding):
  * Nodes sharded 12500/core (padded 12544). Each core computes its shard of
    every h; AllGather replicates h0/h1 (bf16) so each core can gather
    arbitrary-source rows for its destination edges.
  * Aggregation: edges bucketed host-side by (dst-tile of 256, relation) into
    128-edge chunks. One batched indirect DMA per 4-tile group gathers
    h[src] rows (bf16, 256B/row). Per chunk, one fused DVE tensor_scalar
    (is_equal + mult against a row-iota) builds a degree-weighted one-hot
    [128e x 256d]; one PE matmul (lhsT=G[e,feat], rhs=onehot) accumulates
    means directly in [feat, dst] PSUM layout. No scatter, no transposes.
  * Layer matmul: out[dst,hid] += aggT/selfT [feat,dst-slice].T @ W block.
    Self features arrive via DMA-transpose of the bf16 h shard.
  * Classifier: rhs-major (out1T[hid,dst] = Wcls1.T @ catT), then y^T
    PE-transposed back to [dst, 10]; log_softmax on a packed [128, 980] tile.
"""

import math
import os
import sys

import numpy as np

if "/opt/trn_rl_repo" not in sys.path:
    sys.path.insert(0, "/opt/trn_rl_repo")

import ml_dtypes

BF16 = ml_dtypes.bfloat16

P = 128  # partitions / chunk size
W = 256  # dst-tile width (bf16 iota exact up to 256)


class Cfg:
    def __init__(self, N=100000, K=8, E=100000, NFEAT=500, NHID=128, NCLS=10,
                 LAYERS=2, NCORES=8, TPG=4):
        self.N, self.K, self.E = N, K, E
        self.NFEAT, self.NHID, self.NCLS, self.LAYERS = NFEAT, NHID, NCLS, LAYERS
        self.NCORES, self.TPG = NCORES, TPG
        assert N % NCORES == 0
        self.SHARD = N // NCORES
        self.SPAD = ((self.SHARD + W - 1) // W) * W      # padded shard (12544)
        self.NT = self.SPAD // W                          # dst tiles (49)
        self.NT2 = self.SPAD // P                         # 128-subtiles (98)
        self.NG = (self.NT + TPG - 1) // TPG              # gather groups (13)
        self.NFP = ((NFEAT + 1 + P - 1) // P) * P         # padded feat+bias (512)
        self.NFC = self.NFP // P                          # feat chunks (4)


def _prep(inputs, cfg):
    """Host-side sharding/pack. Returns (in_maps, meta)."""
    c = cfg
    x = np.asarray(inputs["x"], np.float32)
    edges = np.asarray(inputs["edges"]).astype(np.int64)
    W_in = np.asarray(inputs["W_in"], np.float32)
    b_in = np.asarray(inputs["b_in"], np.float32)
    W_conv = np.asarray(inputs["W_conv"], np.float32)
    b_conv = np.asarray(inputs["b_conv"], np.float32)
    W_cls1 = np.asarray(inputs["W_cls1"], np.float32)
    b_cls1 = np.asarray(inputs["b_cls1"], np.float32)
    W_cls2 = np.asarray(inputs["W_cls2"], np.float32)
    b_cls2 = np.asarray(inputs["b_cls2"], np.float32)

    src = edges[:, 0, :]
    dst = edges[:, 1, :]

    # degree-normalization weights (graph preprocessing)
    wvals = np.empty((c.K, c.E), np.float32)
    for k in range(c.K):
        deg = np.bincount(dst[k], minlength=c.N).astype(np.float32)
        wvals[k] = 1.0 / np.maximum(deg[dst[k]], 1.0)

    phi_src = (src // c.SHARD) * c.SPAD + (src % c.SHARD)   # h_full row ids
    core_of = dst // c.SHARD
    dst_loc = dst - core_of * c.SHARD
    t_of = dst_loc // W
    off_of = dst_loc % W

    # shared chunk structure: C[t,k] = max over cores of ceil(max(cnt,1)/128)
    cnt = np.zeros((c.NCORES, c.NT, c.K), np.int64)
    for k in range(c.K):
        flat = core_of[k] * c.NT + t_of[k]
        cnt[:, :, k] += np.bincount(flat, minlength=c.NCORES * c.NT).reshape(
            c.NCORES, c.NT)
    C = np.ceil(np.maximum(cnt, 1) / P).max(axis=0).astype(np.int64)  # [NT, K]

    # column layout, group-major: for g, for t in g, for k: C[t,k] chunks
    colstart = np.zeros((c.NT, c.K), np.int64)
    gcol = []  # per group (c0, c1)
    col = 0
    for g in range(c.NG):
        g0 = col
        for t in range(g * c.TPG, min((g + 1) * c.TPG, c.NT)):
            for k in range(c.K):
                colstart[t, k] = col
                col += C[t, k]
        gcol.append((g0, col))
    TOTCOL = col
    COLS_MAX = max(c1 - c0 for c0, c1 in gcol)

    # per-core edge arrays
    eidx = np.zeros((c.NCORES, P, TOTCOL), np.int32)
    eoff = np.full((c.NCORES, P, TOTCOL), -1.0, np.float32)
    ew = np.zeros((c.NCORES, P, TOTCOL), np.float32)
    for cc in range(c.NCORES):
        for k in range(c.K):
            m = core_of[k] == cc
            tt = t_of[k][m]
            order = np.argsort(tt, kind="stable")
            tt = tt[order]
            oo = off_of[k][m][order]
            gg = phi_src[k][m][order]
            ww = wvals[k][m][order]
            start_of_t = np.searchsorted(tt, np.arange(c.NT))
            pos = np.arange(len(tt)) - start_of_t[tt]
            cols = colstart[tt, k] + pos // P
            rows = pos % P
            eidx[cc, rows, cols] = gg
            eoff[cc, rows, cols] = oo.astype(np.float32)
            ew[cc, rows, cols] = ww

    # x^T tiles, padded: [NFC, NT2, 128, 128] per core
    xt = np.zeros((c.NCORES, c.NFC, c.NT2, P, P), BF16)
    for cc in range(c.NCORES):
        xs = x[cc * c.SHARD:(cc + 1) * c.SHARD]           # [SHARD, NFEAT]
        xa = np.zeros((c.NFP, c.SPAD), np.float32)
        xa[:c.NFEAT, :c.SHARD] = xs.T
        xa[c.NFEAT, :c.SHARD] = 1.0                        # bias row
        xt[cc] = (
            xa.reshape(c.NFC, P, c.NT2, P).transpose(0, 2, 1, 3).astype(BF16))

    # weights (replicated)
    wia = np.zeros((c.NFP, c.NHID), np.float32)
    wia[:c.NFEAT] = W_in
    wia[c.NFEAT] = b_in
    win = wia.reshape(c.NFC, P, c.NHID).astype(BF16)

    nb = c.K + 1
    wcv = np.zeros((c.LAYERS, nb + 1, P, c.NHID), np.float32)
    for l in range(c.LAYERS):
        wcv[l, :nb] = W_conv[l].reshape(nb, c.NHID, c.NHID)
        wcv[l, nb, 0, :] = b_conv[l]                       # bias block (row 0)
    wcv = wcv.astype(BF16)

    ncat = c.LAYERS + 1
    wc1 = W_cls1.reshape(ncat, c.NHID, c.NHID).astype(BF16)

    meta = dict(C=C, colstart=colstart, gcol=gcol, TOTCOL=TOTCOL,
                COLS_MAX=COLS_MAX)
    in_maps = []
    for cc in range(c.NCORES):
        in_maps.append({
            "xt": np.ascontiguousarray(xt[cc]),
            "eidx": np.ascontiguousarray(eidx[cc]),
            "eoff": np.ascontiguousarray(eoff[cc]),
            "ew": np.ascontiguousarray(ew[cc]),
            "win": np.ascontiguousarray(win),
            "wconv": np.ascontiguousarray(wcv),
            "wcls1": np.ascontiguousarray(wc1),
            "bcls1": np.ascontiguousarray(b_cls1.reshape(1, c.NHID).astype(BF16)),
            "wcls2": np.ascontiguousarray(W_cls2.astype(BF16)),
            "bcls2": np.ascontiguousarray(b_cls2.reshape(1, c.NCLS).astype(BF16)),
        })
    return in_maps, meta


def _build(cfg, meta):
    from concourse import bacc, bass, mybir, tile
    from concourse.bass import IndirectOffsetOnAxis
    from concourse.masks import make_identity

    c = cfg
    C, colstart, gcol = meta["C"], meta["colstart"], meta["gcol"]
    TOTCOL, COLS_MAX = meta["TOTCOL"], meta["COLS_MAX"]
    f32, bf16, i32 = mybir.dt.float32, mybir.dt.bfloat16, mybir.dt.int32
    AF = mybir.ActivationFunctionType
    OP = mybir.AluOpType

    nc = bacc.Bacc("TRN2", target_bir_lowering=False, debug=False,
                   enable_asserts=False, num_devices=c.NCORES)

    # DRAM I/O
    d_xt = nc.dram_tensor("xt", [c.NFC, c.NT2, P, P], bf16, kind="ExternalInput")
    d_eidx = nc.dram_tensor("eidx", [P, TOTCOL], i32, kind="ExternalInput")
    d_eoff = nc.dram_tensor("eoff", [P, TOTCOL], f32, kind="ExternalInput")
    d_ew = nc.dram_tensor("ew", [P, TOTCOL], f32, kind="ExternalInput")
    d_win = nc.dram_tensor("win", [c.NFC, P, c.NHID], bf16, kind="ExternalInput")
    d_wconv = nc.dram_tensor("wconv", [c.LAYERS, c.K + 2, P, c.NHID], bf16,
                             kind="ExternalInput")
    d_wcls1 = nc.dram_tensor("wcls1", [c.LAYERS + 1, P, c.NHID], bf16,
                             kind="ExternalInput")
    d_bcls1 = nc.dram_tensor("bcls1", [1, c.NHID], bf16, kind="ExternalInput")
    d_wcls2 = nc.dram_tensor("wcls2", [c.NHID, c.NCLS], bf16,
                             kind="ExternalInput")
    d_bcls2 = nc.dram_tensor("bcls2", [1, c.NCLS], bf16, kind="ExternalInput")
    d_out = nc.dram_tensor("out", [P, c.NT2 * c.NCLS], f32,
                           kind="ExternalOutput")

    h_loc = [nc.dram_tensor(f"h{i}_loc", [c.SPAD, c.NHID], bf16,
                            kind="Internal") for i in range(c.LAYERS + 1)]
    h_full = [nc.dram_tensor(f"h{i}_full", [c.SPAD * c.NCORES, c.NHID], bf16,
                             kind="Internal", addr_space="Shared")
              for i in range(c.LAYERS)]
    groups = [[0]] if c.NCORES == 1 else [list(range(c.NCORES))]

    with tile.TileContext(nc) as tc:
        import contextlib
        with contextlib.ExitStack() as ctx:
            pc = ctx.enter_context(tc.tile_pool(name="const", bufs=1))
            ps = ctx.enter_context(tc.tile_pool(name="sb", bufs=2))
            pg = ctx.enter_context(tc.tile_pool(name="gb", bufs=2))
            poh = ctx.enter_context(tc.tile_pool(name="ohp", bufs=4))
            pp = ctx.enter_context(tc.tile_pool(name="psA", bufs=1, space="PSUM"))
            pm = ctx.enter_context(tc.tile_pool(name="psB", bufs=2, space="PSUM"))

            # ---- constants / persistent
            iota_i = pc.tile([P, W], i32, tag="iota_i")
            nc.gpsimd.iota(iota_i[:], pattern=[[1, W]], base=0,
                           channel_multiplier=0)
            iota_b = pc.tile([P, W], bf16, tag="iota_b")
            nc.vector.tensor_copy(iota_b[:], iota_i[:])
            ones = pc.tile([1, W], bf16, tag="ones")
            nc.vector.memset(ones[:], 1.0)
            ident = pc.tile([P, P], f32, tag="ident")
            make_identity(nc, ident[:])

            eidx_s = pc.tile([P, TOTCOL], i32, tag="eidx")
            nc.sync.dma_start(eidx_s[:], d_eidx[:, :])
            eoff_s = pc.tile([P, TOTCOL], f32, tag="eoff")
            nc.sync.dma_start(eoff_s[:], d_eoff[:, :])
            ew_s = pc.tile([P, TOTCOL], f32, tag="ew")
            nc.sync.dma_start(ew_s[:], d_ew[:, :])

            NH = c.NHID
            win_s = pc.tile([P, c.NFC * NH], bf16, tag="win")
            for ci in range(c.NFC):
                nc.sync.dma_start(win_s[:, ci * NH:(ci + 1) * NH],
                                  d_win[ci, :, :])

            def win_sl(ci):
                return win_s[:, ci * NH:(ci + 1) * NH]

            NB = c.K + 2
            wconv_s = pc.tile([P, c.LAYERS * NB * NH], bf16, tag="wconv")
            for l in range(c.LAYERS):
                for b in range(NB):
                    j = (l * NB + b) * NH
                    nc.sync.dma_start(wconv_s[:, j:j + NH], d_wconv[l, b, :, :])

            def wconv_sl(l, b, prows=P):
                j = (l * NB + b) * NH
                return wconv_s[:prows, j:j + NH]

            wcls1_s = pc.tile([P, (c.LAYERS + 1) * NH], bf16, tag="wcls1")
            for i in range(c.LAYERS + 1):
                nc.sync.dma_start(wcls1_s[:, i * NH:(i + 1) * NH],
                                  d_wcls1[i, :, :])

            def wcls1_sl(i):
                return wcls1_s[:, i * NH:(i + 1) * NH]

            bcls1_s = pc.tile([1, c.NHID], bf16, tag="bcls1")
            nc.sync.dma_start(bcls1_s[:], d_bcls1[:, :])
            wcls2_s = pc.tile([c.NHID, c.NCLS], bf16, tag="wcls2")
            nc.sync.dma_start(wcls2_s[:], d_wcls2[:, :])
            bcls2_s = pc.tile([1, c.NCLS], bf16, tag="bcls2")
            nc.sync.dma_start(bcls2_s[:], d_bcls2[:, :])

            # ---- input projection: h0 = x @ W_in + b_in (bias folded)
            for t2 in range(c.NT2):
                pmm = pm.tile([P, c.NHID], f32, tag="mm")
                for ci in range(c.NFC):
                    xtile = ps.tile([P, P], bf16, tag="xt")
                    nc.sync.dma_start(xtile[:], d_xt[ci, t2, :, :])
                    nc.tensor.matmul(pmm[:], lhsT=xtile[:], rhs=win_sl(ci),
                                     start=(ci == 0), stop=(ci == c.NFC - 1))
                hob = ps.tile([P, c.NHID], bf16, tag="hout")
                nc.scalar.copy(hob[:], pmm[:])
                nc.sync.dma_start(h_loc[0][t2 * P:(t2 + 1) * P, :], hob[:])

            nc.gpsimd.collective_compute(
                "AllGather", mybir.AluOpType.bypass, replica_groups=groups,
                ins=[h_loc[0][:, :]], outs=[h_full[0][:, :]])

            # ---- conv layers
            for l in range(c.LAYERS):
                hf, hl, hn = h_full[l], h_loc[l], h_loc[l + 1]
                for g in range(c.NG):
                    c0, c1 = gcol[g]
                    gt = pg.tile([P, COLS_MAX, P], bf16, tag="gbuf")
                    # HW indirect DMA only honors [P,1] offset APs — one
                    # gather instruction per 128-edge chunk column.
                    for col in range(c0, c1):
                        nc.gpsimd.indirect_dma_start(
                            out=gt[:, col - c0, :], out_offset=None,
                            in_=hf[:, :],
                            in_offset=IndirectOffsetOnAxis(
                                ap=eidx_s[:, col:col + 1], axis=0))
                    for t in range(g * c.TPG, min((g + 1) * c.TPG, c.NT)):
                        pair = [pp.tile([P, 2 * W], f32, tag=f"pp{j}",
                                        name=f"pair{j}")
                                for j in range(c.K // 2)]
                        for k in range(c.K):
                            cs = colstart[t, k]
                            po = pair[k // 2][:, (k % 2) * W:(k % 2 + 1) * W]
                            for ch in range(C[t, k]):
                                col = cs + ch
                                oh = poh.tile([P, W], bf16, tag="oh")
                                nc.vector.tensor_scalar(
                                    out=oh[:], in0=iota_b[:],
                                    scalar1=eoff_s[:, col:col + 1],
                                    scalar2=ew_s[:, col:col + 1],
                                    op0=OP.is_equal, op1=OP.mult)
                                nc.tensor.matmul(
                                    po, lhsT=gt[:, col - c0, :], rhs=oh[:],
                                    start=(ch == 0), stop=(ch == C[t, k] - 1))
                        aggT = []
                        for j in range(c.K // 2):
                            ab = ps.tile([P, 2 * W], bf16, tag=f"agg{j}")
                            eng = nc.scalar if j % 2 == 0 else nc.vector
                            if j % 2 == 0:
                                nc.scalar.copy(ab[:], pair[j][:])
                            else:
                                nc.vector.tensor_copy(ab[:], pair[j][:])
                            aggT.append(ab)
                        selfT = ps.tile([P, W], bf16, tag="selfT")
                        nc.sync.dma_start(
                            selfT[:], hl[t * W:(t + 1) * W, :], transpose=True)
                        for s in range(2):
                            sl = slice(s * P, (s + 1) * P)
                            pmm = pm.tile([P, c.NHID], f32, tag="mm")
                            nc.tensor.matmul(pmm[:], lhsT=selfT[:, sl],
                                             rhs=wconv_sl(l, 0),
                                             start=True, stop=False)
                            for k in range(c.K):
                                a = aggT[k // 2][:, (k % 2) * W + s * P:
                                                 (k % 2) * W + (s + 1) * P]
                                nc.tensor.matmul(pmm[:], lhsT=a,
                                                 rhs=wconv_sl(l, k + 1),
                                                 start=False, stop=False)
                            nc.tensor.matmul(pmm[:], lhsT=ones[:1, :P],
                                             rhs=wconv_sl(l, c.K + 1, prows=1),
                                             start=False, stop=True)
                            hob = ps.tile([P, c.NHID], bf16, tag="hout")
                            nc.scalar.activation(hob[:], pmm[:], AF.Relu)
                            nc.sync.dma_start(
                                hn[t * W + s * P:t * W + (s + 1) * P, :],
                                hob[:])
                if l + 1 < c.LAYERS:
                    nc.gpsimd.collective_compute(
                        "AllGather", mybir.AluOpType.bypass,
                        replica_groups=groups,
                        ins=[hn[:, :]], outs=[h_full[l + 1][:, :]])

            # ---- classifier + log_softmax
            smbuf = pc.tile([P, c.NT2 * c.NCLS], f32, tag="smbuf")
            for t in range(c.NT):
                catT = []
                for i in range(c.LAYERS + 1):
                    ct = ps.tile([P, W], bf16, tag=f"cat{i}")
                    nc.sync.dma_start(ct[:], h_loc[i][t * W:(t + 1) * W, :],
                                      transpose=True)
                    catT.append(ct)
                ps1 = pp.tile([P, W], f32, tag="pp0")
                for i in range(c.LAYERS + 1):
                    nc.tensor.matmul(ps1[:], lhsT=wcls1_sl(i),
                                     rhs=catT[i][:], start=(i == 0), stop=False)
                nc.tensor.matmul(ps1[:], lhsT=bcls1_s[:1, :], rhs=ones[:1, :W],
                                 start=False, stop=True)
                r1 = ps.tile([P, W], bf16, tag="r1")
                nc.scalar.activation(r1[:], ps1[:], AF.Relu)
                ps2 = pp.tile([c.NCLS, W], f32, tag="pp1")
                nc.tensor.matmul(ps2[:], lhsT=wcls2_s[:, :], rhs=r1[:],
                                 start=True, stop=False)
                nc.tensor.matmul(ps2[:], lhsT=bcls2_s[:1, :], rhs=ones[:1, :W],
                                 start=False, stop=True)
                yT = ps.tile([c.NCLS, W], f32, tag="yT")
                nc.scalar.copy(yT[:], ps2[:])
                for s in range(2):
                    ptr = pp.tile([P, c.NCLS], f32, tag="pp2")
                    nc.tensor.transpose(ptr[:], yT[:, s * P:(s + 1) * P],
                                        ident[:c.NCLS, :c.NCLS])
                    j = (2 * t + s) * c.NCLS
                    nc.scalar.copy(smbuf[:, j:j + c.NCLS], ptr[:])

            smv = smbuf[:].rearrange("p (n q) -> p n q", q=c.NCLS)
            mx = pc.tile([P, c.NT2, 1], f32, tag="mx")
            nc.vector.tensor_reduce(mx[:, :, 0], smv, axis=mybir.AxisListType.X,
                                    op=OP.max)
            xm = pc.tile([P, c.NT2 * c.NCLS], f32, tag="xm")
            xmv = xm[:].rearrange("p (n q) -> p n q", q=c.NCLS)
            nc.vector.tensor_tensor(
                out=xmv, in0=smv, in1=mx[:].to_broadcast([P, c.NT2, c.NCLS]),
                op=OP.subtract)
            ex = pc.tile([P, c.NT2 * c.NCLS], f32, tag="ex")
            nc.scalar.activation(ex[:], xm[:], AF.Exp)
            sm = pc.tile([P, c.NT2, 1], f32, tag="sm")
            nc.vector.tensor_reduce(
                sm[:, :, 0], ex[:].rearrange("p (n q) -> p n q", q=c.NCLS),
                axis=mybir.AxisListType.X, op=OP.add)
            ls = pc.tile([P, c.NT2, 1], f32, tag="ls")
            nc.scalar.activation(ls[:], sm[:], AF.Ln)
            ob = pc.tile([P, c.NT2 * c.NCLS], f32, tag="ob")
            nc.vector.tensor_tensor(
                out=ob[:].rearrange("p (n q) -> p n q", q=c.NCLS), in0=xmv,
                in1=ls[:].to_broadcast([P, c.NT2, c.NCLS]), op=OP.subtract)
            nc.sync.dma_start(d_out[:, :], ob[:])

    nc.compile()
    return nc


_CACHE = {}


def _get(cfg, inputs):
    key = (hash(np.asarray(inputs["edges"]).tobytes()),
           np.asarray(inputs["x"]).shape)
    if key not in _CACHE:
        in_maps, meta = _prep(inputs, cfg)
        nc = _build(cfg, meta)
        _CACHE[key] = (nc, meta)
    else:
        nc, meta = _CACHE[key]
        in_maps, _ = _prep(inputs, cfg)
    return nc, in_maps, meta


def _unshard(results, cfg):
    c = cfg
    shards = []
    for cc in range(c.NCORES):
        o = results[cc]["out"]                          # [128, NT2*NCLS]
        o = o.reshape(P, c.NT2, c.NCLS).transpose(1, 0, 2).reshape(
            c.SPAD, c.NCLS)
        shards.append(o[:c.SHARD])
    return np.concatenate(shards, axis=0).astype(np.float32)


def kernel(**inputs) -> np.ndarray:
    from concourse import bass_utils
    cfg = Cfg()
    nc, in_maps, _ = _get(cfg, inputs)
    res = bass_utils.run_bass_kernel_spmd(
        nc, in_maps, core_ids=list(range(cfg.NCORES)))
    return _unshard(res.results, cfg)
